# revision 18
# baseline (speedup 1.0000x reference)
"""BandSplitModule Trainium2 kernel.

Math (per band k with c=2w channels, folding layernorm affine + linear):
  out[n,t] = invstd[t] * sum_c X[c,t]*W2[c,n] + v[n]
where
  W2[c,n] = g[c]*W[c,n] - mean_c'(g*W)[n]     (removes the mean term)
  v[n]    = sum_c b[c]*W[c,n] + cbias[n]
  invstd[t] = 1/sqrt(E[X^2] - E[X]^2 + eps)
The invstd multiply is folded into the matmul by pre-scaling X columns:
  Xs[c,t] = X[c,t] * invstd[t]  ->  out = W2^T @ Xs + v.

Per core: one batch element. Bands are packed into 16 "super-tiles" of
128 partitions ([4 x c32] | [2 x c64] | [1 x c128]); column sums (stats)
are computed with ones-matmuls accumulating into per-(group,chunk) PSUM
tiles; invstd is broadcast across partitions with a selector matmul.
All matmuls run in float32r (single-pass fp32, ~1e-3 rel err).
"""
import numpy as np

B, F, T = 8, 1025, 2048
NF = 128                       # features
EPS = 1e-8
CHUNK = 512
NCH = T // CHUNK               # 4

# (start_bin, width, n_bands) per group; c = 2*w channels per band
GROUP_DEFS = [(0, 16, 16), (256, 32, 8), (512, 64, 8)]

_cache = {}
FANCY_XLOAD = False


def _supertiles():
    """Returns list of groups; each group: dict with c, K (bands in group),
    band list of super-tiles: each super-tile: list of
    (global_band, idx_in_group, part_off, real_row0)."""
    groups = []
    gb = 0
    for gi, (s, w, nb) in enumerate(GROUP_DEFS):
        c = 2 * w
        per_st = 128 // c
        sts = []
        for st0 in range(0, nb, per_st):
            bands = []
            for j in range(per_st):
                bi = st0 + j
                bands.append((gb + bi, bi, j * c, s + bi * w))
            sts.append(bands)
        groups.append(dict(gi=gi, c=c, w=w, K=nb, sts=sts))
        gb += nb
    return groups


def _precompute(inputs):
    """Host-side folded weights, selectors, ones matrices (float64 math)."""
    groups = _supertiles()
    n_st = sum(len(g["sts"]) for g in groups)
    w2 = np.zeros((n_st, 128, NF), np.float32)
    vmat = np.zeros((128, 32), np.float32)
    ones2 = np.zeros((n_st, 2, 128, 64), np.float32)   # [st][which][part][col]
    sel = np.zeros((n_st, 16, 128), np.float32)
    tags = ("16", "32", "64")
    sti = 0
    for g in groups:
        gi, c, K = g["gi"], g["c"], g["K"]
        tag = tags[gi]
        gg = np.asarray(inputs["g" + tag], np.float64)
        bb = np.asarray(inputs["b" + tag], np.float64)
        WW = np.asarray(inputs["W" + tag], np.float64)
        cc = np.asarray(inputs["c" + tag], np.float64)
        for bands in g["sts"]:
            for (gband, ig, off, _r0) in bands:
                Wg = gg[ig][:, None] * WW[ig]            # (c, NF)
                W2b = Wg - Wg.mean(axis=0, keepdims=True)
                w2[sti, off:off + c, :] = W2b.astype(np.float32)
                vmat[:, gband] = (bb[ig] @ WW[ig] + cc[ig]).astype(np.float32)
                ones2[sti, 0, off:off + c, ig] = 1.0
                ones2[sti, 1, off:off + c, 32 + ig] = 1.0
                sel[sti, ig, off:off + c] = 1.0
            sti += 1
    return dict(w2=w2, vmat=vmat, ones2=ones2, sel=sel)


def _build_nc():
    import concourse.bass as bass
    import concourse.tile as tile
    from concourse import mybir

    f32 = mybir.dt.float32
    f32r = mybir.dt.float32r
    AF = mybir.ActivationFunctionType
    ALU = mybir.AluOpType

    groups = _supertiles()
    n_st = sum(len(g["sts"]) for g in groups)

    nc = bass.Bass("TRN2", debug=False)
    xr = nc.dram_tensor("x_real", [F, T], f32, kind="ExternalInput").ap()
    xi = nc.dram_tensor("x_imag", [F, T], f32, kind="ExternalInput").ap()
    w2d = nc.dram_tensor("w2", [n_st, 128, NF], f32, kind="ExternalInput").ap()
    vd = nc.dram_tensor("vmat", [128, 32], f32, kind="ExternalInput").ap()
    onesd = nc.dram_tensor("ones2", [n_st, 2, 128, 64], f32, kind="ExternalInput").ap()
    seld = nc.dram_tensor("sel", [n_st, 16, 128], f32, kind="ExternalInput").ap()
    outd = nc.dram_tensor("out", [128, 32, T], f32, kind="ExternalOutput").ap()

    with tile.TileContext(nc) as tc:
        with tc.tile_pool(name="consts", bufs=1) as consts, \
             tc.tile_pool(name="xp", bufs=11) as xp, \
             tc.tile_pool(name="x2p", bufs=6) as x2p, \
             tc.tile_pool(name="arp", bufs=5) as arp, \
             tc.tile_pool(name="cmp", bufs=2) as cmp_, \
             tc.tile_pool(name="outp", bufs=6) as outp, \
             tc.tile_pool(name="ps_stats", bufs=2, space="PSUM") as ps_stats, \
             tc.tile_pool(name="ps_a", bufs=2, space="PSUM") as ps_a, \
             tc.tile_pool(name="ps_main", bufs=4, space="PSUM") as ps_main:

            # ---- constants ----
            w2t, onesAt, onesBt, selt = [], [], [], []
            for st in range(n_st):
                t = consts.tile([128, NF], f32r, tag=f"w2_{st}")
                nc.sync.dma_start(out=t[:], in_=w2d[st, :, :].bitcast(f32r))
                w2t.append(t)
                ta = consts.tile([128, 64], f32r, tag=f"onA_{st}")
                nc.sync.dma_start(out=ta[:], in_=onesd[st, 0, :, :].bitcast(f32r))
                onesAt.append(ta)
                tb = consts.tile([128, 64], f32r, tag=f"onB_{st}")
                nc.sync.dma_start(out=tb[:], in_=onesd[st, 1, :, :].bitcast(f32r))
                onesBt.append(tb)
                ts_ = consts.tile([16, 128], f32r, tag=f"sel_{st}")
                nc.sync.dma_start(out=ts_[:], in_=seld[st, :, :].bitcast(f32r))
                selt.append(ts_)
            vt = consts.tile([128, 32], f32, tag="vmat")
            nc.sync.dma_start(out=vt[:], in_=vd[:])
            epst = consts.tile([32, 1], f32, tag="eps")
            nc.vector.memset(epst[:], EPS)

            # ---- per group ----
            sti0 = 0
            for g in groups:
                c, w, K = g["c"], g["w"], g["K"]
                sts = g["sts"]
                nst = len(sts)
                nb = 128 // c
                inv_c = 1.0 / c

                # load X super-tiles: one DMA for real rows, one for imag.
                # DRAM rows for the st's bands are contiguous ([nb*w, T]);
                # band j lands at partition j*c (+w for imag).
                xts = []
                for si, bands in enumerate(sts):
                    xt = xp.tile([128, T], f32r, tag="X")
                    if FANCY_XLOAD:
                        r0 = bands[0][3]
                        xv = xt[:].rearrange("(a b) t -> a b t", a=nb)
                        nc.sync.dma_start(
                            out=xv[:, 0:w, :],
                            in_=xr[r0:r0 + nb * w, :].rearrange(
                                "(a b) t -> a b t", a=nb).bitcast(f32r))
                        nc.sync.dma_start(
                            out=xv[:, w:2 * w, :],
                            in_=xi[r0:r0 + nb * w, :].rearrange(
                                "(a b) t -> a b t", a=nb).bitcast(f32r))
                    else:
                        for (_gb, _ig, off, r0) in bands:
                            nc.sync.dma_start(out=xt[off:off + w, :],
                                              in_=xr[r0:r0 + w, :].bitcast(f32r))
                            nc.sync.dma_start(out=xt[off + w:off + 2 * w, :],
                                              in_=xi[r0:r0 + w, :].bitcast(f32r))
                    xts.append(xt)

                # stats + invstd per chunk
                ars = []
                for ch in range(NCH):
                    cs = slice(ch * CHUNK, (ch + 1) * CHUNK)
                    stats = ps_stats.tile([64, CHUNK], f32, tag="stats")
                    for si in range(nst):
                        sq = x2p.tile([128, CHUNK], f32r, tag="X2")
                        nc.scalar.activation(sq[:], xts[si][:, cs], AF.Square)
                        nc.tensor.matmul(stats[:], onesAt[sti0 + si][:, 0:64],
                                         xts[si][:, cs],
                                         start=(si == 0), stop=False,
                                         skip_group_check=True)
                        nc.tensor.matmul(stats[:], onesBt[sti0 + si][:, 0:64],
                                         sq[:],
                                         start=False, stop=(si == nst - 1),
                                         skip_group_check=True)
                    mu = cmp_.tile([K, CHUNK], f32, tag="mu")
                    nc.vector.tensor_scalar(mu[:], stats[0:K, :], inv_c, None,
                                            ALU.mult)
                    m2 = cmp_.tile([K, CHUNK], f32, tag="m2")
                    nc.vector.tensor_mul(m2[:], mu[:], mu[:])
                    varr = cmp_.tile([K, CHUNK], f32, tag="varr")
                    nc.vector.scalar_tensor_tensor(varr[:], stats[32:32 + K, :],
                                                   inv_c, m2[:],
                                                   ALU.mult, ALU.subtract)
                    sd = cmp_.tile([K, CHUNK], f32, tag="sd")
                    nc.scalar.activation(sd[:], varr[:], AF.Sqrt,
                                         bias=epst[0:K, 0:1])
                    ar = arp.tile([K, CHUNK], f32r, tag="ar")
                    with nc.allow_low_precision(reason="f32r tag only; fp32 bits"):
                        nc.vector.reciprocal(ar[:], sd[:])
                    ars.append(ar)

                # scale + project, one whole-band output DMA per band
                for si, bands in enumerate(sts):
                    ots = [outp.tile([128, T], f32, tag="O", name=f"ot{si}_{bj}")
                           for bj in range(len(bands))]
                    for ch in range(NCH):
                        cs = slice(ch * CHUNK, (ch + 1) * CHUNK)
                        at = ps_a.tile([128, CHUNK], f32, tag="A")
                        nc.tensor.matmul(at[:], selt[sti0 + si][0:K, :],
                                         ars[ch][:], start=True, stop=True)
                        nc.vector.tensor_mul(xts[si][:, cs], xts[si][:, cs],
                                             at[:].bitcast(f32r))
                        for bj, (gband, _ig, off, _r0) in enumerate(bands):
                            pm = ps_main.tile([128, CHUNK], f32, tag="M")
                            nc.tensor.matmul(pm[:],
                                             w2t[sti0 + si][off:off + c, :],
                                             xts[si][off:off + c, cs],
                                             start=True, stop=True,
                                             tile_position=(off, 0))
                            nc.scalar.activation(ots[bj][:, cs], pm[:],
                                                 AF.Identity,
                                                 bias=vt[:, gband:gband + 1])
                    for bj, (gband, _ig, off, _r0) in enumerate(bands):
                        nc.sync.dma_start(out=outd[:, gband, :],
                                           in_=ots[bj][:])
                sti0 += nst
    return nc


def _split_excess_waits(nc, max_waits=1):
    """This walrus build rejects >1 semaphore wait on compute-instruction
    templates, while Tile freely attaches several. Hoist all but one wait
    onto standalone InstEventSemaphore instructions inserted just before,
    on the same engine — semantically identical (AND of ge-waits, engine
    stalls in program order)."""
    import concourse.mybir as mybir

    counter = 0
    for f in nc.m.functions:
        for blk in f.blocks:
            new_list = []
            changed = False
            for ins in blk.instructions:
                si = ins.sync_info
                ow = list(si.on_wait) if si is not None and si.on_wait else []
                if (
                    len(ow) > max_waits
                    and type(ins).__name__ != "InstEventSemaphore"
                    and all(w.wait_mode == "sem-ge-imm" for w in ow)
                ):
                    for w in ow[:-max_waits]:
                        ev = mybir.InstEventSemaphore(
                            name=f"evwait_split_{counter}", ins=[], outs=[]
                        )
                        counter += 1
                        ev.engine = ins.engine
                        ev.bass_nofuse = True
                        ev.debug = ins.debug
                        ev.sync_info = mybir.SyncInfo(on_wait=[w], on_update=[])
                        new_list.append(ev)
                    ins.sync_info = mybir.SyncInfo(
                        on_wait=ow[-max_waits:],
                        on_update=list(si.on_update) if si.on_update else [],
                    )
                    changed = True
                new_list.append(ins)
            if changed:
                blk.instructions = new_list
    return counter


def _get_nc():
    if "nc" not in _cache:
        nc = _build_nc()
        _split_excess_waits(nc)
        _cache["nc"] = nc
    return _cache["nc"]


def kernel(**inputs):
    from concourse.bass_utils import run_bass_kernel_spmd

    consts = _precompute(inputs)
    x_real = np.ascontiguousarray(np.asarray(inputs["x_real"], np.float32))
    x_imag = np.ascontiguousarray(np.asarray(inputs["x_imag"], np.float32))

    in_maps = []
    for b in range(B):
        in_maps.append({
            "x_real": x_real[b], "x_imag": x_imag[b],
            "w2": consts["w2"], "vmat": consts["vmat"],
            "ones2": consts["ones2"], "sel": consts["sel"],
        })
    nc = _get_nc()
    res = run_bass_kernel_spmd(nc, in_maps, list(range(B)))
    out = np.stack([res.results[b]["out"] for b in range(B)], axis=0)
    return out
